# revision 7
# baseline (speedup 1.0000x reference)
"""Trainium2 Bass kernel for nn_CrossAttention_82471962018390.

Dilated (d=2) 9x9 neighborhood cross-attention, q 48x48 vs k/v 24x24.

Math identity used: the nearest-exact 2x upsample + dilation-2 NATTEN window
collapses so that query (h, w) attends to the ORIGINAL 24x24 k/v grid at
rows clip(h//2-4, 0, 15) + 0..8, cols clip(w//2-4, 0, 15) + 0..8 (a
contiguous 9x9 window; the 4 queries in each 2x2 block share one window).

Kernel structure (per (b, head) pair; 2 pairs per core, 8 cores = 16 pairs):
  - 16 row-bands by s_h = clip(h//2-4,0,15); band s attends the 9x24=216-key
    slab k[:, s:s+9, :].
  - Scores computed transposed: S^T[key, query] = (K slab)^T @ Q, with the
    column-window mask folded INTO the matmul via 16 extra contraction rows.
    scale 1/8 is folded into q on the host.  Both key chunks (128 + 88) land
    in ONE [128, 1024] PSUM tile (chunk2 at column offset 512).
  - exp on ScalarE (PSUM -> SBUF bf16), one instruction per chunk.
  - PV computed TRANSPOSED: out[query, dh] with lhsT = exp-scores (queries
    become PSUM partitions) and rhs = [V | ones-column], so the softmax
    denominator lands as a per-partition scalar in column 64.  Normalize is
    then one tiny strided reciprocal ([q, nblocks]) plus ONE broadcast
    tensor-multiply per group -- this removed a 30 us DVE reciprocal chain.
  - Output DMA'd per 96/120-query block as [q, 64] rows; host transposes.
The issue order is software-pipelined: QK runs two iterations ahead of PV.
"""

import numpy as np
import ml_dtypes

try:
    import concourse.bass as bass
    import concourse.bacc as bacc
    import concourse.tile as tile
    from concourse import mybir
    from concourse.bass_utils import run_bass_kernel_spmd
except ImportError:  # pragma: no cover
    import sys

    sys.path.insert(0, "/opt/trn_rl_repo")
    import concourse.bass as bass
    import concourse.bacc as bacc
    import concourse.tile as tile
    from concourse import mybir
    from concourse.bass_utils import run_bass_kernel_spmd

from contextlib import ExitStack

BF = ml_dtypes.bfloat16
N_CORES = 8
NPAIR = 2  # (b, head) pairs per core
DH = 64
HQ = WQ = 48
HK = WK = 24
NQ = HQ * WQ  # 2304
NK = HK * WK  # 576
BAND_KEYS = 9 * WK  # 216
CH1 = 128  # keys in chunk 1 of a band
CH2 = BAND_KEYS - CH1  # 88
C2OFF = 512  # column offset of chunk-2 scores inside the PSUM score tile

# s(i) = clip(i//2 - 4, 0, 15) for i in 0..47
_S = np.clip(np.arange(48) // 2 - 4, 0, 15)

# Band groups: (q column offset, width, [(band s, rel q offset, band width)])
# band s covers h rows where s_h(h) == s: s=0 -> h 0..9, s=1..14 -> 2 rows,
# s=15 -> h 38..47.
def _groups():
    bands_h0 = {}
    for h in range(48):
        bands_h0.setdefault(int(_S[h]), []).append(h)
    spans = {s: (hs[0], len(hs)) for s, hs in bands_h0.items()}
    layout = [[0], [1, 2, 3, 4, 5], [6, 7, 8, 9, 10], [11, 12, 13, 14], [15]]
    groups = []
    for g in layout:
        h0 = spans[g[0]][0]
        width = sum(spans[s][1] for s in g) * 48
        bands = []
        off = 0
        for s in g:
            bw = spans[s][1] * 48
            bands.append((s, off, bw))
            off += bw
        groups.append((h0 * 48, width, bands))
    return groups


GROUPS = _groups()
MAXM = max(w for _, w, _ in GROUPS)  # 480

# PV blocks per group: (query col start, n queries, band index within group).
# Interior bands are one 96-query block; the 480-query edge bands split into
# four 120-query blocks (PSUM partitions cap out tiles at 128 queries).
def _blocks(gi):
    bands = GROUPS[gi][2]
    blocks = []
    for bi, (s, off, bw) in enumerate(bands):
        if bw <= 128:
            blocks.append((off, bw, bi))
        else:
            for t in range(bw // 120):
                blocks.append((off + 120 * t, 120, bi))
    return blocks


BLOCKS = [_blocks(gi) for gi in range(len(GROUPS))]
NBMAX = max(len(b) for b in BLOCKS)  # 5


def _host_tables():
    """M0exp [16, 576] and Bw [16, 2304] mask/one-hot tables (fp32)."""
    m0 = np.full((16, WK), -30.0, np.float32)
    for r in range(16):
        m0[r, r : r + 9] = 0.0
    m0exp = np.tile(m0[:, None, :], (1, HK, 1)).reshape(16, NK)
    bw = np.zeros((16, NQ), np.float32)
    for w in range(48):
        bw[_S[w], np.arange(48) * 48 + w] = 1.0
    return m0exp, bw


def build_kernel(ctx: ExitStack, tc, qb, km, vb, out):
    nc = tc.nc
    FP32 = mybir.dt.float32
    BF16 = mybir.dt.bfloat16
    Exp = mybir.ActivationFunctionType.Exp

    qpool = ctx.enter_context(tc.tile_pool(name="qb", bufs=1))
    kpool = ctx.enter_context(tc.tile_pool(name="km", bufs=1))
    vpool = ctx.enter_context(tc.tile_pool(name="vt", bufs=1))
    spool = ctx.enter_context(tc.tile_pool(name="scores", bufs=1, space="PSUM"))
    opool = ctx.enter_context(tc.tile_pool(name="opsum", bufs=1, space="PSUM"))
    epool = ctx.enter_context(tc.tile_pool(name="expo", bufs=1))
    rpool = ctx.enter_context(tc.tile_pool(name="res", bufs=1))

    ITERS = [(p, gi) for p in range(NPAIR) for gi in range(len(GROUPS))]
    NI = len(ITERS)

    # Persistent double-buffered tiles, managed explicitly for pipelining.
    sT = [spool.tile([128, 1024], FP32, name=f"s{j}") for j in range(2)]
    eT = [epool.tile([128, 992], BF16, name=f"e{j}") for j in range(2)]
    # PV out: [query, block, 64 pv + 1 sumexp] -- 325 cols, fits one bank.
    oT = [opool.tile([128, NBMAX, 65], FP32, name=f"o{j}") for j in range(2)]
    rcpT = [rpool.tile([128, 8], FP32, name=f"rcp{j}") for j in range(2)]
    resT = [rpool.tile([128, NBMAX, 64], FP32, name=f"res{j}") for j in range(2)]

    # Rotating persistent V' moving tiles: cols 0:64 = V slab chunk (DMA'd
    # per band), col 64 = ones (memset once; gives the sumexp column).
    NVT = 24
    vtiles = [vpool.tile([128, 65], BF16, name=f"vt{j}") for j in range(NVT)]
    for vt in vtiles:
        nc.gpsimd.memset(vt[:, 64:65], 1.0)

    # Per-pair inputs: km in one DMA; qb split per group so QK(0) starts early.
    km_t, qb_t = [], []
    for p in range(NPAIR):
        kt = kpool.tile([80, NK], BF16, name=f"km{p}")
        nc.sync.dma_start(kt[:, :], km[80 * p : 80 * p + 80, :])
        km_t.append(kt)
        qt = qpool.tile([80, NQ], BF16, name=f"qb{p}")
        qb_t.append(qt)
    for gi, (q0, M, _) in enumerate(GROUPS):
        for p in range(NPAIR):
            nc.sync.dma_start(
                qb_t[p][:, q0 : q0 + M], qb[80 * p : 80 * p + 80, q0 : q0 + M]
            )

    # Pre-assign rotating V tiles per iteration (deterministic round-robin):
    # one (vta, vtb) pair per *band*.
    vmap = []
    vct = 0
    for p, gi in ITERS:
        pairs = []
        for _ in GROUPS[gi][2]:
            pairs.append((vtiles[vct % NVT], vtiles[(vct + 1) % NVT]))
            vct += 2
        vmap.append(pairs)

    def vdma(i):
        p, gi = ITERS[i]
        for (s, _, _), (vta, vtb) in zip(GROUPS[gi][2], vmap[i]):
            row0 = (p * 16 + s) * BAND_KEYS
            nc.sync.dma_start(vta[:, 0:64], vb[row0 : row0 + CH1, :])
            nc.sync.dma_start(vtb[0:CH2, 0:64], vb[row0 + CH1 : row0 + BAND_KEYS, :])

    def qk(i):
        p, gi = ITERS[i]
        q0, M, bands = GROUPS[gi]
        s = sT[i % 2]
        for (b, off, bw) in bands:
            rhs = qb_t[p][:, q0 + off : q0 + off + bw]
            nc.tensor.matmul(
                s[:, off : off + bw],
                km_t[p][:, 24 * b : 24 * b + CH1],
                rhs,
                start=True,
                stop=True,
            )
            nc.tensor.matmul(
                s[0:CH2, C2OFF + off : C2OFF + off + bw],
                km_t[p][:, 24 * b + CH1 : 24 * b + BAND_KEYS],
                rhs,
                start=True,
                stop=True,
            )

    def expi(i):
        _, gi = ITERS[i]
        M = GROUPS[gi][1]
        s, e = sT[i % 2], eT[i % 2]
        # Two instructions: an ACT read must not cross a PSUM bank boundary.
        nc.scalar.activation(e[:, 0:M], s[:, 0:M], Exp)
        nc.scalar.activation(
            e[0:CH2, C2OFF : C2OFF + M], s[0:CH2, C2OFF : C2OFF + M], Exp
        )

    def pv(i):
        p, gi = ITERS[i]
        o = oT[i % 2]
        e = eT[i % 2]
        for j, (cs, qn, bi) in enumerate(BLOCKS[gi]):
            vta, vtb = vmap[i][bi]
            nc.tensor.matmul(
                o[0:qn, j, :],
                e[:, cs : cs + qn],
                vta[:, :],
                start=True,
                stop=False,
            )
            nc.tensor.matmul(
                o[0:qn, j, :],
                e[0:CH2, C2OFF + cs : C2OFF + cs + qn],
                vtb[0:CH2, :],
                start=False,
                stop=True,
            )

    def norm(i):
        p, gi = ITERS[i]
        q0 = GROUPS[gi][0]
        blocks = BLOCKS[gi]
        nb = len(blocks)
        qn = blocks[0][1]  # uniform within a group (96 or 120)
        o, rcp, res = oT[i % 2], rcpT[i % 2], resT[i % 2]
        nc.vector.reciprocal(rcp[0:qn, 0:nb], o[0:qn, 0:nb, 64])
        nc.vector.tensor_mul(
            res[0:qn, 0:nb, :],
            o[0:qn, 0:nb, 0:64],
            rcp[0:qn, 0:nb].broadcast_to((qn, nb, 64)),
        )
        for j, (cs, _, _) in enumerate(blocks):
            nc.sync.dma_start(
                out[NQ * p + q0 + cs : NQ * p + q0 + cs + qn, :], res[0:qn, j, :]
            )

    # Software-pipelined issue order: PE stays two iterations ahead on QK.
    vdma(0)
    vdma(1)
    qk(0)
    qk(1)
    expi(0)
    for i in range(NI):
        pv(i)
        if i + 2 < NI:
            vdma(i + 2)
            qk(i + 2)
        if i + 1 < NI:
            expi(i + 1)
        norm(i)


_CACHE = {}


def _get_nc():
    if "nc" not in _CACHE:
        nc = bacc.Bacc(
            "TRN2", target_bir_lowering=False, debug=False, num_devices=N_CORES
        )
        qb = nc.dram_tensor(
            "qb", [NPAIR * 80, NQ], mybir.dt.bfloat16, kind="ExternalInput"
        ).ap()
        km = nc.dram_tensor(
            "km", [NPAIR * 80, NK], mybir.dt.bfloat16, kind="ExternalInput"
        ).ap()
        vb = nc.dram_tensor(
            "vb", [NPAIR * 16 * BAND_KEYS, DH], mybir.dt.bfloat16, kind="ExternalInput"
        ).ap()
        out = nc.dram_tensor(
            "out", [NPAIR * NQ, DH], mybir.dt.float32, kind="ExternalOutput"
        ).ap()
        with tile.TileContext(nc) as tc, ExitStack() as ctx:
            build_kernel(ctx, tc, qb, km, vb, out)
        nc.compile()
        _CACHE["nc"] = nc
    return _CACHE["nc"]


def kernel(q: np.ndarray, k: np.ndarray, v: np.ndarray) -> np.ndarray:
    assert q.shape == (2, 512, HQ, WQ) and k.shape == (2, 512, HK, WK)
    m0exp, bw = _host_tables()
    nc = _get_nc()

    in_maps = []
    for c in range(N_CORES):
        qbc = np.empty((NPAIR * 80, NQ), BF)
        kmc = np.empty((NPAIR * 80, NK), BF)
        vbc = np.empty((NPAIR * 16 * BAND_KEYS, DH), BF)
        for pl in range(NPAIR):
            pg = NPAIR * c + pl
            b, hd = pg // 8, pg % 8
            qbc[80 * pl : 80 * pl + 64] = (
                q[b, 64 * hd : 64 * hd + 64].reshape(64, NQ) / 8.0
            ).astype(BF)
            qbc[80 * pl + 64 : 80 * pl + 80] = bw.astype(BF)
            kmc[80 * pl : 80 * pl + 64] = (
                k[b, 64 * hd : 64 * hd + 64].reshape(64, NK).astype(BF)
            )
            kmc[80 * pl + 64 : 80 * pl + 80] = m0exp.astype(BF)
            v3 = v[b, 64 * hd : 64 * hd + 64].reshape(64, HK, WK)
            for s in range(16):
                row0 = (pl * 16 + s) * BAND_KEYS
                vbc[row0 : row0 + BAND_KEYS] = (
                    v3[:, s : s + 9, :].reshape(64, BAND_KEYS).T.astype(BF)
                )
        in_maps.append({"qb": qbc, "km": kmc, "vb": vbc})

    results = run_bass_kernel_spmd(nc, in_maps, list(range(N_CORES))).results

    out = np.empty((2, 512, HQ, WQ), np.float32)
    for c in range(N_CORES):
        o = results[c]["out"]  # [NPAIR*2304, 64], rows = flat query h*48+w
        for pl in range(NPAIR):
            pg = NPAIR * c + pl
            b, hd = pg // 8, pg % 8
            out[b, 64 * hd : 64 * hd + 64] = (
                o[NQ * pl : NQ * pl + NQ].T.reshape(64, HQ, WQ)
            )
    return out


if __name__ == "__main__":
    qq = np.load("/root/problem/q.npy")
    kk = np.load("/root/problem/k.npy")
    vv = np.load("/root/problem/v.npy")
    got = kernel(qq, kk, vv)
    exp = np.load("/root/problem/expected.npy")
    rel = np.linalg.norm(got - exp) / np.linalg.norm(exp)
    print("Relative error:", rel)


# revision 9
# speedup vs baseline: 1.0119x; 1.0119x over previous
"""Trainium2 Bass kernel for nn_CrossAttention_82471962018390.

Dilated (d=2) 9x9 neighborhood cross-attention, q 48x48 vs k/v 24x24.

Math identity used: the nearest-exact 2x upsample + dilation-2 NATTEN window
collapses so that query (h, w) attends to the ORIGINAL 24x24 k/v grid at
rows clip(h//2-4, 0, 15) + 0..8, cols clip(w//2-4, 0, 15) + 0..8 (a
contiguous 9x9 window; the 4 queries in each 2x2 block share one window).

Kernel structure (per (b, head) pair; 2 pairs per core, 8 cores = 16 pairs):
  - 16 row-bands by s_h = clip(h//2-4,0,15); band s attends the 9x24=216-key
    slab k[:, s:s+9, :].
  - Scores computed transposed: S^T[key, query] = (K slab)^T @ Q, with the
    column-window mask folded INTO the matmul via 16 extra contraction rows.
    scale 1/8 is folded into q on the host.  Both key chunks (128 + 88) land
    in ONE [128, 1024] PSUM tile (chunk2 at column offset 512).
  - exp on ScalarE (PSUM -> SBUF bf16), one instruction per chunk.
  - PV computed TRANSPOSED: out[query, dh] with lhsT = exp-scores (queries
    become PSUM partitions) and rhs = [V | ones-column], so the softmax
    denominator lands as a per-partition scalar in column 64.  Normalize is
    then one tiny strided reciprocal ([q, nblocks]) plus ONE broadcast
    tensor-multiply per group -- this removed a 30 us DVE reciprocal chain.
  - Output DMA'd per 96/120-query block as [q, 64] rows; host transposes.
The issue order is software-pipelined: QK runs two iterations ahead of PV.
"""

import numpy as np
import ml_dtypes

try:
    import concourse.bass as bass
    import concourse.bacc as bacc
    import concourse.tile as tile
    from concourse import mybir
    from concourse.bass_utils import run_bass_kernel_spmd
except ImportError:  # pragma: no cover
    import sys

    sys.path.insert(0, "/opt/trn_rl_repo")
    import concourse.bass as bass
    import concourse.bacc as bacc
    import concourse.tile as tile
    from concourse import mybir
    from concourse.bass_utils import run_bass_kernel_spmd

from contextlib import ExitStack

BF = ml_dtypes.bfloat16
N_CORES = 8
NPAIR = 2  # (b, head) pairs per core
DH = 64
HQ = WQ = 48
HK = WK = 24
NQ = HQ * WQ  # 2304
NK = HK * WK  # 576
BAND_KEYS = 9 * WK  # 216
CH1 = 128  # keys in chunk 1 of a band
CH2 = BAND_KEYS - CH1  # 88
C2OFF = 512  # column offset of chunk-2 scores inside the PSUM score tile

# s(i) = clip(i//2 - 4, 0, 15) for i in 0..47
_S = np.clip(np.arange(48) // 2 - 4, 0, 15)

# Band groups: (q column offset, width, [(band s, rel q offset, band width)])
# band s covers h rows where s_h(h) == s: s=0 -> h 0..9, s=1..14 -> 2 rows,
# s=15 -> h 38..47.
def _groups():
    bands_h0 = {}
    for h in range(48):
        bands_h0.setdefault(int(_S[h]), []).append(h)
    spans = {s: (hs[0], len(hs)) for s, hs in bands_h0.items()}
    layout = [[0], [1, 2, 3, 4, 5], [6, 7, 8, 9, 10], [11, 12, 13, 14], [15]]
    groups = []
    for g in layout:
        h0 = spans[g[0]][0]
        width = sum(spans[s][1] for s in g) * 48
        bands = []
        off = 0
        for s in g:
            bw = spans[s][1] * 48
            bands.append((s, off, bw))
            off += bw
        groups.append((h0 * 48, width, bands))
    return groups


GROUPS = _groups()
MAXM = max(w for _, w, _ in GROUPS)  # 480

# PV blocks per group: (query col start, n queries, band index within group).
# Interior bands are one 96-query block; the 480-query edge bands split into
# four 120-query blocks (PSUM partitions cap out tiles at 128 queries).
def _blocks(gi):
    bands = GROUPS[gi][2]
    blocks = []
    for bi, (s, off, bw) in enumerate(bands):
        if bw <= 128:
            blocks.append((off, bw, bi))
        else:
            for t in range(bw // 120):
                blocks.append((off + 120 * t, 120, bi))
    return blocks


BLOCKS = [_blocks(gi) for gi in range(len(GROUPS))]
NBMAX = max(len(b) for b in BLOCKS)  # 5


def _host_tables():
    """M0exp [16, 576] and Bw [16, 2304] mask/one-hot tables (fp32)."""
    m0 = np.full((16, WK), -30.0, np.float32)
    for r in range(16):
        m0[r, r : r + 9] = 0.0
    m0exp = np.tile(m0[:, None, :], (1, HK, 1)).reshape(16, NK)
    bw = np.zeros((16, NQ), np.float32)
    for w in range(48):
        bw[_S[w], np.arange(48) * 48 + w] = 1.0
    return m0exp, bw


def build_kernel(ctx: ExitStack, tc, qb, km, vb, out):
    nc = tc.nc
    FP32 = mybir.dt.float32
    BF16 = mybir.dt.bfloat16
    Exp = mybir.ActivationFunctionType.Exp

    qpool = ctx.enter_context(tc.tile_pool(name="qb", bufs=1))
    kpool = ctx.enter_context(tc.tile_pool(name="km", bufs=1))
    vpool = ctx.enter_context(tc.tile_pool(name="vt", bufs=1))
    spool = ctx.enter_context(tc.tile_pool(name="scores", bufs=1, space="PSUM"))
    opool = ctx.enter_context(tc.tile_pool(name="opsum", bufs=1, space="PSUM"))
    epool = ctx.enter_context(tc.tile_pool(name="expo", bufs=1))
    rpool = ctx.enter_context(tc.tile_pool(name="res", bufs=1))

    ITERS = [(p, gi) for p in range(NPAIR) for gi in range(len(GROUPS))]
    NI = len(ITERS)

    # Persistent multi-buffered tiles, managed explicitly for pipelining.
    # Three score buffers so QK(i+2) never waits on EXP(i) freeing a tile.
    sT = [spool.tile([128, 1024], FP32, name=f"s{j}") for j in range(3)]
    eT = [epool.tile([128, 992], BF16, name=f"e{j}") for j in range(2)]
    # PV out: [query, block, 64 pv + 1 sumexp] -- 325 cols, fits one bank.
    oT = [opool.tile([128, NBMAX, 65], FP32, name=f"o{j}") for j in range(2)]
    rcpT = [rpool.tile([128, 8], FP32, name=f"rcp{j}") for j in range(2)]
    resT = [rpool.tile([128, NBMAX, 64], FP32, name=f"res{j}") for j in range(2)]

    # Rotating persistent V' moving tiles: cols 0:64 = V slab chunk (DMA'd
    # per band), col 64 = ones (memset once; gives the sumexp column).
    NVT = 32
    vtiles = [vpool.tile([128, 65], BF16, name=f"vt{j}") for j in range(NVT)]
    for vt in vtiles:
        nc.gpsimd.memset(vt[:, 64:65], 1.0)

    # Per-pair inputs: km in one DMA; qb split per group so QK(0) starts early.
    km_t, qb_t = [], []
    for p in range(NPAIR):
        kt = kpool.tile([80, NK], BF16, name=f"km{p}")
        nc.sync.dma_start(kt[:, :], km[80 * p : 80 * p + 80, :])
        km_t.append(kt)
        qt = qpool.tile([80, NQ], BF16, name=f"qb{p}")
        qb_t.append(qt)
    for gi, (q0, M, _) in enumerate(GROUPS):
        for p in range(NPAIR):
            nc.sync.dma_start(
                qb_t[p][:, q0 : q0 + M], qb[80 * p : 80 * p + 80, q0 : q0 + M]
            )

    # Pre-assign rotating V tiles per iteration (deterministic round-robin):
    # one (vta, vtb) pair per *band*.
    vmap = []
    vct = 0
    for p, gi in ITERS:
        pairs = []
        for _ in GROUPS[gi][2]:
            pairs.append((vtiles[vct % NVT], vtiles[(vct + 1) % NVT]))
            vct += 2
        vmap.append(pairs)

    def vdma(i):
        p, gi = ITERS[i]
        for (s, _, _), (vta, vtb) in zip(GROUPS[gi][2], vmap[i]):
            row0 = (p * 16 + s) * BAND_KEYS
            nc.sync.dma_start(vta[:, 0:64], vb[row0 : row0 + CH1, :])
            nc.sync.dma_start(vtb[0:CH2, 0:64], vb[row0 + CH1 : row0 + BAND_KEYS, :])

    def qk(i):
        p, gi = ITERS[i]
        q0, M, bands = GROUPS[gi]
        s = sT[i % 3]
        for (b, off, bw) in bands:
            rhs = qb_t[p][:, q0 + off : q0 + off + bw]
            nc.tensor.matmul(
                s[:, off : off + bw],
                km_t[p][:, 24 * b : 24 * b + CH1],
                rhs,
                start=True,
                stop=True,
            )
            nc.tensor.matmul(
                s[0:CH2, C2OFF + off : C2OFF + off + bw],
                km_t[p][:, 24 * b + CH1 : 24 * b + BAND_KEYS],
                rhs,
                start=True,
                stop=True,
            )

    def expi(i):
        _, gi = ITERS[i]
        M = GROUPS[gi][1]
        s, e = sT[i % 3], eT[i % 2]
        # Two instructions: an ACT read must not cross a PSUM bank boundary.
        nc.scalar.activation(e[:, 0:M], s[:, 0:M], Exp)
        nc.scalar.activation(
            e[0:CH2, C2OFF : C2OFF + M], s[0:CH2, C2OFF : C2OFF + M], Exp
        )

    def pv(i):
        p, gi = ITERS[i]
        o = oT[i % 2]
        e = eT[i % 2]
        for j, (cs, qn, bi) in enumerate(BLOCKS[gi]):
            vta, vtb = vmap[i][bi]
            nc.tensor.matmul(
                o[0:qn, j, :],
                e[:, cs : cs + qn],
                vta[:, :],
                start=True,
                stop=False,
            )
            nc.tensor.matmul(
                o[0:qn, j, :],
                e[0:CH2, C2OFF + cs : C2OFF + cs + qn],
                vtb[0:CH2, :],
                start=False,
                stop=True,
            )

    def norm(i):
        p, gi = ITERS[i]
        q0 = GROUPS[gi][0]
        blocks = BLOCKS[gi]
        nb = len(blocks)
        qn = blocks[0][1]  # uniform within a group (96 or 120)
        o, rcp, res = oT[i % 2], rcpT[i % 2], resT[i % 2]
        nc.vector.reciprocal(rcp[0:qn, 0:nb], o[0:qn, 0:nb, 64])
        nc.vector.tensor_mul(
            res[0:qn, 0:nb, :],
            o[0:qn, 0:nb, 0:64],
            rcp[0:qn, 0:nb].broadcast_to((qn, nb, 64)),
        )
        for j, (cs, _, _) in enumerate(blocks):
            nc.sync.dma_start(
                out[NQ * p + q0 + cs : NQ * p + q0 + cs + qn, :], res[0:qn, j, :]
            )

    # Software-pipelined issue order: PE stays two iterations ahead on QK.
    vdma(0)
    vdma(1)
    qk(0)
    qk(1)
    expi(0)
    for i in range(NI):
        if i + 2 < NI:
            vdma(i + 2)
            qk(i + 2)
        pv(i)
        if i + 1 < NI:
            expi(i + 1)
        norm(i)


_CACHE = {}


def _get_nc():
    if "nc" not in _CACHE:
        nc = bacc.Bacc(
            "TRN2", target_bir_lowering=False, debug=False, num_devices=N_CORES
        )
        qb = nc.dram_tensor(
            "qb", [NPAIR * 80, NQ], mybir.dt.bfloat16, kind="ExternalInput"
        ).ap()
        km = nc.dram_tensor(
            "km", [NPAIR * 80, NK], mybir.dt.bfloat16, kind="ExternalInput"
        ).ap()
        vb = nc.dram_tensor(
            "vb", [NPAIR * 16 * BAND_KEYS, DH], mybir.dt.bfloat16, kind="ExternalInput"
        ).ap()
        out = nc.dram_tensor(
            "out", [NPAIR * NQ, DH], mybir.dt.float32, kind="ExternalOutput"
        ).ap()
        with tile.TileContext(nc) as tc, ExitStack() as ctx:
            build_kernel(ctx, tc, qb, km, vb, out)
        nc.compile()
        _CACHE["nc"] = nc
    return _CACHE["nc"]


def kernel(q: np.ndarray, k: np.ndarray, v: np.ndarray) -> np.ndarray:
    assert q.shape == (2, 512, HQ, WQ) and k.shape == (2, 512, HK, WK)
    m0exp, bw = _host_tables()
    nc = _get_nc()

    in_maps = []
    for c in range(N_CORES):
        qbc = np.empty((NPAIR * 80, NQ), BF)
        kmc = np.empty((NPAIR * 80, NK), BF)
        vbc = np.empty((NPAIR * 16 * BAND_KEYS, DH), BF)
        for pl in range(NPAIR):
            pg = NPAIR * c + pl
            b, hd = pg // 8, pg % 8
            qbc[80 * pl : 80 * pl + 64] = (
                q[b, 64 * hd : 64 * hd + 64].reshape(64, NQ) / 8.0
            ).astype(BF)
            qbc[80 * pl + 64 : 80 * pl + 80] = bw.astype(BF)
            kmc[80 * pl : 80 * pl + 64] = (
                k[b, 64 * hd : 64 * hd + 64].reshape(64, NK).astype(BF)
            )
            kmc[80 * pl + 64 : 80 * pl + 80] = m0exp.astype(BF)
            v3 = v[b, 64 * hd : 64 * hd + 64].reshape(64, HK, WK)
            for s in range(16):
                row0 = (pl * 16 + s) * BAND_KEYS
                vbc[row0 : row0 + BAND_KEYS] = (
                    v3[:, s : s + 9, :].reshape(64, BAND_KEYS).T.astype(BF)
                )
        in_maps.append({"qb": qbc, "km": kmc, "vb": vbc})

    results = run_bass_kernel_spmd(nc, in_maps, list(range(N_CORES))).results

    out = np.empty((2, 512, HQ, WQ), np.float32)
    for c in range(N_CORES):
        o = results[c]["out"]  # [NPAIR*2304, 64], rows = flat query h*48+w
        for pl in range(NPAIR):
            pg = NPAIR * c + pl
            b, hd = pg // 8, pg % 8
            out[b, 64 * hd : 64 * hd + 64] = (
                o[NQ * pl : NQ * pl + NQ].T.reshape(64, HQ, WQ)
            )
    return out


if __name__ == "__main__":
    qq = np.load("/root/problem/q.npy")
    kk = np.load("/root/problem/k.npy")
    vv = np.load("/root/problem/v.npy")
    got = kernel(qq, kk, vv)
    exp = np.load("/root/problem/expected.npy")
    rel = np.linalg.norm(got - exp) / np.linalg.norm(exp)
    print("Relative error:", rel)
